# revision 4
# baseline (speedup 1.0000x reference)
"""Trainium2 Bass kernel for nn_Att_3_layer1 (dense attention scorer + masked softmax).

Math (per batch b):
    v_proj = relu(v @ Wv^T + bv)                    [O, H]
    q_proj = relu(q @ Wq^T + bq)                    [KT, H]
    joint  = v_proj[o] * q_proj[kt]  (elementwise)  per (kt, o) pair
    pre    = joint @ Wn^T + bn                      [KT, O, G]
    logits = relu(pre) @ Wl^T + bl                  [KT, O]
    w      = softmax(where(mask, logits, -1e9), axis=o)

Sharding: data-parallel over batch (B=8) across 8 NeuronCores; weights replicated.

Device-side algorithm (per core, one batch element):
  - All matmuls run in fp32r (1 cycle/row on the PE at N>=256, ~1e-4 rel err).
  - Stage 1: v_projT[h, o] = relu(Wv @ v^T + bv), o padded to 256 for full-rate fp32r.
  - Stage 2: q_projT[h, kt] = relu(Wq @ q^T + bq).
  - Per o (100 iterations): J_o[h, kt] = q_projT * v_projT[:, o] (DVE per-partition
    scalar mul, output rounded to fp32r); then pre[kt, g] = J_o^T @ Wn'^T as 8
    accumulating 128x128x512 fp32r matmuls.
  - Wl contraction trick: relu(x)*wl summed over g == sum(relu(x*|wl|)*sign(wl)).
    |Wl| is folded into Wn's rows on the host and g is permuted so positive-sign
    columns come first; the logits reduce to two relu+sum passes over column
    blocks of PSUM, done with activation(Relu, accum_out=...) on ScalarE (3 of 4
    blocks) and tensor_scalar(max0, accum_out=...) on VectorE (1 of 4), so the
    reduction runs in parallel with the PE matmul stream.
  - bl drops out (softmax shift invariance). bn is zero in this problem instance;
    if nonzero it is folded in via an extra K=1 ones-row matmul.
  - Masked softmax over o: mask converted host-side to additive 0/-1e9, broadcast
    to 128 partitions with a K=1 ones matmul; rowmax via tensor_reduce(negate),
    exp+sum in one activation(Exp, accum_out), reciprocal, scale.
"""

import os
import numpy as np

import concourse.bass as bass
import concourse.tile as tile
from concourse import bacc, mybir
from concourse.bass_utils import run_bass_kernel_spmd

B, K, T, O = 8, 4, 64, 100
VD, QD, H = 2048, 768, 512
KT = K * T            # 256
OPAD = 256            # fp32r matmuls need moving dim >= 256 for 1 cycle/row
NEG = -1e9
N_CORES = 8

f32 = mybir.dt.float32
f32r = mybir.dt.float32r

HT = H // 128     # 4
VT = VD // 128    # 16
QT = QD // 128    # 6
MT = KT // 128    # 2

_BUILD_CACHE: dict = {}


def build_nc(gpos: int, use_bn: bool, repeat: int = 1):
    """Build the Bass program. gpos = number of positive-sign Wl columns after
    the host-side permutation; repeat wraps the whole body in a For_i for
    timing runs."""
    key = (gpos, use_bn, repeat)
    if key in _BUILD_CACHE:
        return _BUILD_CACHE[key]

    nc = bacc.Bacc("TRN2", target_bir_lowering=False, debug=True)

    v_d = nc.dram_tensor("vtp", [VD, OPAD], f32r, kind="ExternalInput")
    q_d = nc.dram_tensor("qtp", [QD, KT], f32r, kind="ExternalInput")
    wvt_d = nc.dram_tensor("wvt", [VD, H], f32r, kind="ExternalInput")
    wqt_d = nc.dram_tensor("wqt", [QD, H], f32r, kind="ExternalInput")
    wnt_d = nc.dram_tensor("wnt", [H, H], f32r, kind="ExternalInput")
    bv_d = nc.dram_tensor("bv", [H, 1], f32, kind="ExternalInput")
    bq_d = nc.dram_tensor("bq", [H, 1], f32, kind="ExternalInput")
    mk_d = nc.dram_tensor("maskadd", [1, O], f32, kind="ExternalInput")
    bn_d = nc.dram_tensor("bn2", [1, H], f32r, kind="ExternalInput") if use_bn else None
    w_d = nc.dram_tensor("w", [KT, O], f32, kind="ExternalOutput")
    it_d = nc.dram_tensor("iters", [1, 1], f32, kind="ExternalOutput") if repeat > 1 else None

    Relu = mybir.ActivationFunctionType.Relu
    Exp = mybir.ActivationFunctionType.Exp

    # stage-5 reduce jobs: (mt, lo, hi, kind, engine); dest tile chosen per job
    jobs = []
    for mt in range(MT):
        if gpos > 0:
            jobs.append((mt, 0, gpos, "P"))
        if gpos < H:
            jobs.append((mt, gpos, H, "N"))
    # one job on DVE, rest on ACT (balances engine load; PE stays the bottleneck)
    dve_jobs = {0}

    with tile.TileContext(nc) as tc:
        import contextlib

        with contextlib.ExitStack() as stk:
            wpool = stk.enter_context(tc.tile_pool(name="wpool", bufs=1))
            jpool = stk.enter_context(tc.tile_pool(name="jpool", bufs=3))
            scrpool = stk.enter_context(tc.tile_pool(name="scrpool", bufs=2))

            # persistent tiles (addresses fixed; loaded inside the loop body)
            wvt_s = wpool.tile([128, VT, H], f32r, tag="wvt")
            wqt_s = wpool.tile([128, QT, H], f32r, tag="wqt")
            wnt_s = wpool.tile([128, HT, H], f32r, tag="wnt")
            vt_s = wpool.tile([128, VT, OPAD], f32r, tag="vt")
            qt_s = wpool.tile([128, QT, KT], f32r, tag="qt")
            bv_s = wpool.tile([128, HT, 1], f32, tag="bv")
            bq_s = wpool.tile([128, HT, 1], f32, tag="bq")
            mk_s = wpool.tile([1, O], f32, tag="mk")
            ones1 = wpool.tile([1, 128], f32, tag="ones1")
            v_projT = wpool.tile([128, HT, O], f32, tag="vproj")
            q_projT = wpool.tile([128, HT, KT], f32, tag="qproj")
            m128 = wpool.tile([128, O], f32, tag="m128")
            lgP = [wpool.tile([128, O], f32, tag=f"lgP{mt}", name=f"lgP{mt}") for mt in range(MT)]
            lgN = [wpool.tile([128, O], f32, tag=f"lgN{mt}", name=f"lgN{mt}") for mt in range(MT)]
            if use_bn:
                ones1r = wpool.tile([1, 128], f32r, tag="ones1r")
                bn_s = wpool.tile([1, H], f32r, tag="bn")

            nc.vector.memset(ones1, 1.0)
            if repeat > 1:
                cnt = wpool.tile([1, 1], f32, tag="cnt")
                nc.vector.memset(cnt, 0.0)
            if use_bn:
                nc.gpsimd.memset(ones1r, 1.0)
            if gpos == 0:
                for mt in range(MT):
                    nc.gpsimd.memset(lgP[mt], 0.0)
            if gpos == H:
                for mt in range(MT):
                    nc.gpsimd.memset(lgN[mt], 0.0)

            loop = (
                tc.For_i(
                    0,
                    repeat,
                    1,
                    hint_engines=(
                        mybir.EngineType.PE,
                        mybir.EngineType.Activation,
                        mybir.EngineType.DVE,
                        mybir.EngineType.SP,
                    ),
                )
                if repeat > 1
                else contextlib.nullcontext()
            )
            with loop:
                if repeat > 1:
                    nc.vector.tensor_scalar_add(cnt, cnt, 1.0)
                # ---- input DMAs (ordered so stage-1 deps land first)
                nc.sync.dma_start(out=wvt_s, in_=wvt_d.rearrange("(t p) h -> p t h", p=128))
                nc.sync.dma_start(out=vt_s, in_=v_d.rearrange("(t p) o -> p t o", p=128))
                nc.sync.dma_start(out=wqt_s, in_=wqt_d.rearrange("(t p) h -> p t h", p=128))
                nc.sync.dma_start(out=qt_s, in_=q_d.rearrange("(t p) kt -> p t kt", p=128))
                nc.sync.dma_start(out=wnt_s, in_=wnt_d.rearrange("(t p) g -> p t g", p=128))
                nc.sync.dma_start(out=bv_s, in_=bv_d.rearrange("(t p) x -> p t x", p=128))
                nc.sync.dma_start(out=bq_s, in_=bq_d.rearrange("(t p) x -> p t x", p=128))
                nc.sync.dma_start(out=mk_s, in_=mk_d[:])
                if use_bn:
                    nc.sync.dma_start(out=bn_s, in_=bn_d[:])

                # ---- stage 1 + 2: projections (PSUM pool scoped so the o-loop
                # pool can use all banks afterwards)
                with tc.tile_pool(name="pre_ps", bufs=1, space="PSUM") as pre_ps:
                    for m in range(HT):
                        pv = pre_ps.tile([128, OPAD], f32, tag=f"pv{m}")
                        for k in range(VT):
                            nc.tensor.matmul(
                                pv[:],
                                wvt_s[:, k, m * 128 : (m + 1) * 128],
                                vt_s[:, k, :],
                                start=(k == 0),
                                stop=(k == VT - 1),
                            )
                        nc.scalar.activation(
                            out=v_projT[:, m, :], in_=pv[:, :O], func=Relu,
                            bias=bv_s[:, m, :], scale=1.0,
                        )
                    for m in range(HT):
                        pq = pre_ps.tile([128, KT], f32, tag=f"pq{m}")
                        for k in range(QT):
                            nc.tensor.matmul(
                                pq[:],
                                wqt_s[:, k, m * 128 : (m + 1) * 128],
                                qt_s[:, k, :],
                                start=(k == 0),
                                stop=(k == QT - 1),
                            )
                        nc.scalar.activation(
                            out=q_projT[:, m, :], in_=pq[:], func=Relu,
                            bias=bq_s[:, m, :], scale=1.0,
                        )

                with tc.tile_pool(name="o_ps", bufs=1, space="PSUM") as pso:
                    # mask broadcast to all 128 partitions via K=1 ones matmul
                    pm = pso.tile([128, O], f32, tag="pm")
                    nc.tensor.matmul(pm[:], ones1[:], mk_s[:], start=True, stop=True)
                    nc.vector.tensor_copy(m128, pm[:])

                    # ---- main o-loop
                    for o in range(O):
                        js = [
                            jpool.tile([128, KT], f32r, tag=f"j{k}", name=f"j{k}_{o}")
                            for k in range(HT)
                        ]
                        for k in range(HT):
                            nc.vector.tensor_scalar_mul(
                                js[k], q_projT[:, k, :], v_projT[:, k, o : o + 1]
                            )
                        for mt in range(MT):
                            po = pso.tile([128, H], f32, tag=f"po{mt}", bufs=2)
                            for k in range(HT):
                                nc.tensor.matmul(
                                    po[:],
                                    js[k][:, mt * 128 : (mt + 1) * 128],
                                    wnt_s[:, k, :],
                                    start=(k == 0),
                                    stop=(k == HT - 1 and not use_bn),
                                )
                            if use_bn:
                                nc.tensor.matmul(
                                    po[:], ones1r[:], bn_s[:], start=False, stop=True
                                )
                            for ji, (jmt, lo, hi, kind) in enumerate(jobs):
                                if jmt != mt:
                                    continue
                                dest = (lgP if kind == "P" else lgN)[mt][:, o : o + 1]
                                scr = scrpool.tile(
                                    [128, hi - lo], f32, tag=f"scr{ji}",
                                    name=f"scr{ji}_{o}",
                                )
                                if ji in dve_jobs:
                                    nc.vector.tensor_scalar(
                                        out=scr,
                                        in0=po[:, lo:hi],
                                        scalar1=0.0,
                                        scalar2=None,
                                        op0=mybir.AluOpType.max,
                                        op1=mybir.AluOpType.add,
                                        accum_out=dest,
                                    )
                                else:
                                    nc.scalar.activation(
                                        out=scr, in_=po[:, lo:hi], func=Relu,
                                        accum_out=dest,
                                    )

                # ---- masked softmax over o (free dim), per kt-tile
                with tc.tile_pool(name="soft", bufs=1) as soft:
                    for mt in range(MT):
                        lg = soft.tile([128, O], f32, tag=f"lg{mt}")
                        nc.vector.tensor_sub(lg, lgP[mt], lgN[mt])
                        lgm = soft.tile([128, O], f32, tag=f"lgm{mt}")
                        nc.vector.tensor_add(lgm, lg, m128)
                        mx = soft.tile([128, 1], f32, tag=f"mx{mt}")
                        nc.vector.tensor_reduce(
                            out=mx, in_=lgm, axis=mybir.AxisListType.X,
                            op=mybir.AluOpType.max, negate=True,
                        )
                        e = soft.tile([128, O], f32, tag=f"e{mt}")
                        se = soft.tile([128, 1], f32, tag=f"se{mt}")
                        nc.scalar.activation(
                            out=e, in_=lgm, func=Exp, bias=mx, scale=1.0, accum_out=se
                        )
                        rs = soft.tile([128, 1], f32, tag=f"rs{mt}")
                        nc.vector.reciprocal(rs, se)
                        wt = soft.tile([128, O], f32, tag=f"wt{mt}")
                        nc.vector.tensor_scalar_mul(wt, e, rs)
                        nc.sync.dma_start(
                            out=w_d.rearrange("(mt p) o -> p mt o", p=128)[:, mt, :],
                            in_=wt,
                        )
                    if repeat > 1:
                        nc.sync.dma_start(out=it_d[:], in_=cnt)

    nc.finalize()
    _BUILD_CACHE[key] = nc
    return nc


def prepare_in_maps(v, q, box_mask, Wv, bv, Wq, bq, Wn, bn, Wl, bl):
    """Host-side prep: transposes, |Wl| fold + sign permutation, mask->additive."""
    v = np.ascontiguousarray(np.asarray(v, dtype=np.float32))
    q = np.asarray(q, dtype=np.float32).reshape(B, KT, QD)
    box_mask = np.asarray(box_mask)
    Wv = np.asarray(Wv, dtype=np.float32)
    Wq = np.asarray(Wq, dtype=np.float32)
    Wn = np.asarray(Wn, dtype=np.float32)
    Wl = np.asarray(Wl, dtype=np.float32)
    bv = np.asarray(bv, dtype=np.float32)
    bq = np.asarray(bq, dtype=np.float32)
    bn = np.asarray(bn, dtype=np.float32)
    # bl is dropped: softmax is shift-invariant.

    wl = Wl.reshape(H)
    pos = np.flatnonzero(wl > 0)
    rest = np.flatnonzero(~(wl > 0))
    perm = np.concatenate([pos, rest])
    gpos = int(pos.size)
    wn2t = np.ascontiguousarray((Wn * np.abs(wl)[:, None])[perm].T)  # [h, g']
    use_bn = bool(np.any(bn != 0.0))
    bn2 = np.ascontiguousarray((bn * np.abs(wl))[perm].reshape(1, H))

    wvt = np.ascontiguousarray(Wv.T)  # [VD, H]
    wqt = np.ascontiguousarray(Wq.T)  # [QD, H]
    maskadd = np.where(box_mask > 0, np.float32(0.0), np.float32(NEG)).astype(np.float32)

    in_maps = []
    for b in range(B):
        vtp = np.zeros((VD, OPAD), dtype=np.float32)
        vtp[:, :O] = v[b].T
        qtp = np.ascontiguousarray(q[b].T)
        m = {
            "vtp": vtp,
            "qtp": qtp,
            "wvt": wvt,
            "wqt": wqt,
            "wnt": wn2t,
            "bv": bv.reshape(H, 1),
            "bq": bq.reshape(H, 1),
            "maskadd": np.ascontiguousarray(maskadd[b : b + 1]),
        }
        if use_bn:
            m["bn2"] = bn2
        in_maps.append(m)
    return in_maps, gpos, use_bn


def kernel(**inputs) -> np.ndarray:
    in_maps, gpos, use_bn = prepare_in_maps(
        inputs["v"], inputs["q"], inputs["box_mask"],
        inputs["Wv"], inputs["bv"], inputs["Wq"], inputs["bq"],
        inputs["Wn"], inputs["bn"], inputs["Wl"], inputs["bl"],
    )
    nc = build_nc(gpos, use_bn, repeat=1)
    res = run_bass_kernel_spmd(nc, in_maps, core_ids=list(range(N_CORES)))
    out = np.stack(
        [res.results[b]["w"].reshape(K, T, O) for b in range(B)]
    )
    return np.ascontiguousarray(out.astype(np.float32))


# revision 9
# speedup vs baseline: 948.4929x; 948.4929x over previous
"""Trainium2 Bass kernel for nn_Att_3_layer1 (dense attention scorer + masked softmax).

Math (per batch b):
    v_proj = relu(v @ Wv^T + bv)                    [O, H]
    q_proj = relu(q @ Wq^T + bq)                    [KT, H]
    joint  = v_proj[o] * q_proj[kt]  (elementwise)  per (kt, o) pair
    pre    = joint @ Wn^T + bn                      [KT, O, G]
    logits = relu(pre) @ Wl^T + bl                  [KT, O]
    w      = softmax(where(mask, logits, -1e9), axis=o)

Sharding: data-parallel over batch (B=8) across 8 NeuronCores; weights replicated.

Device-side algorithm (per core, one batch element):
  - All matmuls run in fp32r (1 cycle/row on the PE at N>=256, ~1e-4 rel err).
  - Stage 1: v_projT[h, o] = relu(Wv @ v^T + bv), o padded to 256 for full-rate fp32r.
  - Stage 2: q_projT[h, kt] = relu(Wq @ q^T + bq).
  - Per o (100 iterations): J_o[h, kt] = q_projT * v_projT[:, o] (DVE per-partition
    scalar mul, output rounded to fp32r); then pre[kt, g] = J_o^T @ Wn'^T as 8
    accumulating 128x128x512 fp32r matmuls.
  - Wl contraction trick: relu(x)*wl summed over g == sum(relu(x*|wl|)*sign(wl)).
    |Wl| is folded into Wn's rows on the host and g is permuted so positive-sign
    columns come first; the logits reduce to two relu+sum passes over column
    blocks of PSUM, done with activation(Relu, accum_out=...) on ScalarE (3 of 4
    blocks) and tensor_scalar(max0, accum_out=...) on VectorE (1 of 4), so the
    reduction runs in parallel with the PE matmul stream.
  - bl drops out (softmax shift invariance). bn is zero in this problem instance;
    if nonzero it is folded in via an extra K=1 ones-row matmul.
  - Masked softmax over o: mask converted host-side to additive 0/-1e9, broadcast
    to 128 partitions with a K=1 ones matmul; rowmax via tensor_reduce(negate),
    exp+sum in one activation(Exp, accum_out), reciprocal, scale.
"""

import os
import numpy as np

import concourse.bass as bass
import concourse.tile as tile
from concourse import bacc, mybir
from concourse.bass_utils import run_bass_kernel_spmd

B, K, T, O = 8, 4, 64, 100
VD, QD, H = 2048, 768, 512
KT = K * T            # 256
OPAD = 256            # fp32r matmuls need moving dim >= 256 for 1 cycle/row
NEG = -1e9
N_CORES = 8

f32 = mybir.dt.float32
f32r = mybir.dt.float32r

HT = H // 128     # 4
VT = VD // 128    # 16
QT = QD // 128    # 6
MT = KT // 128    # 2

_BUILD_CACHE: dict = {}


def build_nc(gpos: int, use_bn: bool, repeat: int = 1, *, dve_jobs=(0, 2), scr_bf16=False, po_bufs=2, j_bufs=3, scr_bufs=2, mm_bf16=False, vt_host_pad=False, chunked_wv=True):
    """Build the Bass program. gpos = number of positive-sign Wl columns after
    the host-side permutation; repeat wraps the whole body in a For_i for
    timing runs."""
    key = (gpos, use_bn, repeat, tuple(dve_jobs), scr_bf16, po_bufs, j_bufs, scr_bufs, mm_bf16, vt_host_pad, chunked_wv)
    if key in _BUILD_CACHE:
        return _BUILD_CACHE[key]

    nc = bacc.Bacc("TRN2", target_bir_lowering=False, debug=True)

    v_d = nc.dram_tensor("vtp", [VD, OPAD if vt_host_pad else O], f32r, kind="ExternalInput")
    q_d = nc.dram_tensor("qtp", [QD, KT], f32r, kind="ExternalInput")
    wvt_d = nc.dram_tensor("wvt", [VD, H], f32r, kind="ExternalInput")
    wqt_d = nc.dram_tensor("wqt", [QD, H], f32r, kind="ExternalInput")
    mmdt = mybir.dt.bfloat16 if mm_bf16 else f32r
    wnt_d = nc.dram_tensor("wnt", [H, H], mmdt, kind="ExternalInput")
    bv_d = nc.dram_tensor("bv", [H, 1], f32, kind="ExternalInput")
    bq_d = nc.dram_tensor("bq", [H, 1], f32, kind="ExternalInput")
    mk_d = nc.dram_tensor("maskadd", [1, O], f32, kind="ExternalInput")
    bn_d = nc.dram_tensor("bn2", [1, H], f32r, kind="ExternalInput") if use_bn else None
    w_d = nc.dram_tensor("w", [KT, O], f32, kind="ExternalOutput")
    it_d = nc.dram_tensor("iters", [1, 1], f32, kind="ExternalOutput") if repeat > 1 else None

    Relu = mybir.ActivationFunctionType.Relu
    Exp = mybir.ActivationFunctionType.Exp

    # stage-5 reduce jobs: (mt, lo, hi, kind, engine); dest tile chosen per job
    jobs = []
    for mt in range(MT):
        if gpos > 0:
            jobs.append((mt, 0, gpos, "P"))
        if gpos < H:
            jobs.append((mt, gpos, H, "N"))
    dve_jobs = set(dve_jobs)

    with tile.TileContext(nc) as tc:
        import contextlib

        with contextlib.ExitStack() as stk:
            wpool = stk.enter_context(tc.tile_pool(name="wpool", bufs=1))
            jpool = stk.enter_context(tc.tile_pool(name="jpool", bufs=j_bufs))
            scrpool = stk.enter_context(tc.tile_pool(name="scrpool", bufs=scr_bufs))

            # persistent tiles (addresses fixed; loaded inside the loop body)
            wvt_s = wpool.tile([128, VT, H], f32r, tag="wvt")
            wqt_s = wpool.tile([128, QT, H], f32r, tag="wqt")
            wnt_s = wpool.tile([128, HT, H], mmdt, tag="wnt")
            vt_s = wpool.tile([128, VT, OPAD], f32r, tag="vt")
            qt_s = wpool.tile([128, QT, KT], f32r, tag="qt")
            bv_s = wpool.tile([128, HT, 1], f32, tag="bv")
            bq_s = wpool.tile([128, HT, 1], f32, tag="bq")
            mk_s = wpool.tile([1, O], f32, tag="mk")
            ones1 = wpool.tile([1, 128], f32, tag="ones1")
            v_projT = wpool.tile([128, HT, O], f32, tag="vproj")
            q_projT = wpool.tile([128, HT, KT], f32, tag="qproj")
            m128 = wpool.tile([128, O], f32, tag="m128")
            lgP = [wpool.tile([128, O], f32, tag=f"lgP{mt}", name=f"lgP{mt}") for mt in range(MT)]
            lgN = [wpool.tile([128, O], f32, tag=f"lgN{mt}", name=f"lgN{mt}") for mt in range(MT)]
            if use_bn:
                ones1r = wpool.tile([1, 128], f32r, tag="ones1r")
                bn_s = wpool.tile([1, H], f32r, tag="bn")

            nc.vector.memset(ones1, 1.0)
            if not vt_host_pad:
                for _k in range(VT):
                    nc.gpsimd.memset(vt_s[:, _k, O:OPAD].bitcast(mybir.dt.uint32), 0)
            if repeat > 1:
                cnt = wpool.tile([1, 1], f32, tag="cnt")
                nc.vector.memset(cnt, 0.0)
            if use_bn:
                nc.gpsimd.memset(ones1r, 1.0)
            if gpos == 0:
                for mt in range(MT):
                    nc.gpsimd.memset(lgP[mt], 0.0)
            if gpos == H:
                for mt in range(MT):
                    nc.gpsimd.memset(lgN[mt], 0.0)

            loop = (
                tc.For_i(
                    0,
                    repeat,
                    1,
                    hint_engines=(
                        mybir.EngineType.PE,
                        mybir.EngineType.Activation,
                        mybir.EngineType.DVE,
                        mybir.EngineType.SP,
                    ),
                )
                if repeat > 1
                else contextlib.nullcontext()
            )
            with loop:
                if repeat > 1:
                    nc.vector.tensor_scalar_add(cnt, cnt, 1.0)
                # ---- input DMAs: stage-2 deps first so the PE gets work
                # early; the big WvT load is split into 4 chunks so stage-1
                # matmuls pipeline with the transfer
                nc.sync.dma_start(out=wqt_s, in_=wqt_d.rearrange("(t p) h -> p t h", p=128))
                nc.sync.dma_start(out=qt_s, in_=q_d.rearrange("(t p) kt -> p t kt", p=128))
                nc.sync.dma_start(out=bq_s, in_=bq_d.rearrange("(t p) x -> p t x", p=128))
                nc.sync.dma_start(out=bv_s, in_=bv_d.rearrange("(t p) x -> p t x", p=128))
                if vt_host_pad:
                    nc.sync.dma_start(out=vt_s, in_=v_d.rearrange("(t p) o -> p t o", p=128))
                else:
                    nc.sync.dma_start(out=vt_s[:, :, :O], in_=v_d.rearrange("(t p) o -> p t o", p=128))
                wvt_src = wvt_d.rearrange("(t p) h -> p t h", p=128)
                VCH = 4 if chunked_wv else 1
                for c in range(VCH):
                    nc.sync.dma_start(
                        out=wvt_s[:, c * (VT // VCH) : (c + 1) * (VT // VCH), :],
                        in_=wvt_src[:, c * (VT // VCH) : (c + 1) * (VT // VCH), :],
                    )
                nc.sync.dma_start(out=wnt_s, in_=wnt_d.rearrange("(t p) g -> p t g", p=128))
                nc.sync.dma_start(out=mk_s, in_=mk_d[:])
                if use_bn:
                    nc.sync.dma_start(out=bn_s, in_=bn_d[:])

                # ---- stage 1 + 2: projections (PSUM pool scoped so the o-loop
                # pool can use all banks afterwards)
                with tc.tile_pool(name="pre_ps", bufs=1, space="PSUM") as pre_ps:
                    for m in range(HT):
                        pq = pre_ps.tile([128, KT], f32, tag=f"pq{m}")
                        for k in range(QT):
                            nc.tensor.matmul(
                                pq[:],
                                wqt_s[:, k, m * 128 : (m + 1) * 128],
                                qt_s[:, k, :],
                                start=(k == 0),
                                stop=(k == QT - 1),
                            )
                        nc.scalar.activation(
                            out=q_projT[:, m, :], in_=pq[:], func=Relu,
                            bias=bq_s[:, m, :], scale=1.0,
                        )
                    # stage 1: k-tiles grouped by WvT DMA chunk so matmuls can
                    # start while later chunks are still in flight
                    pvs = [pre_ps.tile([128, OPAD], f32, tag=f"pv{m}", name=f"pv{m}") for m in range(HT)]
                    for c in range(VCH):
                        for m in range(HT):
                            for k in range(c * (VT // VCH), (c + 1) * (VT // VCH)):
                                nc.tensor.matmul(
                                    pvs[m][:],
                                    wvt_s[:, k, m * 128 : (m + 1) * 128],
                                    vt_s[:, k, :],
                                    start=(k == 0),
                                    stop=(k == VT - 1),
                                )
                    for m in range(HT):
                        nc.scalar.activation(
                            out=v_projT[:, m, :], in_=pvs[m][:, :O], func=Relu,
                            bias=bv_s[:, m, :], scale=1.0,
                        )

                with tc.tile_pool(name="o_ps", bufs=1, space="PSUM") as pso:
                    # mask broadcast to all 128 partitions via K=1 ones matmul
                    pm = pso.tile([128, O], f32, tag="pm")
                    nc.tensor.matmul(pm[:], ones1[:], mk_s[:], start=True, stop=True)
                    nc.vector.tensor_copy(m128, pm[:])

                    # ---- main o-loop
                    for o in range(O):
                        js = [
                            jpool.tile([128, KT], mmdt, tag=f"j{k}", name=f"j{k}_{o}")
                            for k in range(HT)
                        ]
                        for k in range(HT):
                            nc.vector.tensor_scalar_mul(
                                js[k], q_projT[:, k, :], v_projT[:, k, o : o + 1]
                            )
                        for mt in range(MT):
                            po = pso.tile([128, H], f32, tag=f"po{mt}", bufs=po_bufs)
                            for k in range(HT):
                                nc.tensor.matmul(
                                    po[:],
                                    js[k][:, mt * 128 : (mt + 1) * 128],
                                    wnt_s[:, k, :],
                                    start=(k == 0),
                                    stop=(k == HT - 1 and not use_bn),
                                )
                            if use_bn:
                                nc.tensor.matmul(
                                    po[:], ones1r[:], bn_s[:], start=False, stop=True
                                )
                            for ji, (jmt, lo, hi, kind) in enumerate(jobs):
                                if jmt != mt:
                                    continue
                                dest = (lgP if kind == "P" else lgN)[mt][:, o : o + 1]
                                scr = scrpool.tile(
                                    [128, hi - lo],
                                    mybir.dt.bfloat16 if scr_bf16 else f32,
                                    tag=f"scr{ji}",
                                    name=f"scr{ji}_{o}",
                                )
                                if ji in dve_jobs:
                                    nc.vector.tensor_scalar(
                                        out=scr,
                                        in0=po[:, lo:hi],
                                        scalar1=0.0,
                                        scalar2=None,
                                        op0=mybir.AluOpType.max,
                                        op1=mybir.AluOpType.add,
                                        accum_out=dest,
                                    )
                                else:
                                    nc.scalar.activation(
                                        out=scr, in_=po[:, lo:hi], func=Relu,
                                        accum_out=dest,
                                    )

                # ---- masked softmax over o (free dim), per kt-tile
                with tc.tile_pool(name="soft", bufs=1) as soft:
                    for mt in range(MT):
                        lg = soft.tile([128, O], f32, tag=f"lg{mt}")
                        nc.vector.tensor_sub(lg, lgP[mt], lgN[mt])
                        lgm = soft.tile([128, O], f32, tag=f"lgm{mt}")
                        nc.vector.tensor_add(lgm, lg, m128)
                        mx = soft.tile([128, 1], f32, tag=f"mx{mt}")
                        nc.vector.tensor_reduce(
                            out=mx, in_=lgm, axis=mybir.AxisListType.X,
                            op=mybir.AluOpType.max, negate=True,
                        )
                        e = soft.tile([128, O], f32, tag=f"e{mt}")
                        se = soft.tile([128, 1], f32, tag=f"se{mt}")
                        nc.scalar.activation(
                            out=e, in_=lgm, func=Exp, bias=mx, scale=1.0, accum_out=se
                        )
                        rs = soft.tile([128, 1], f32, tag=f"rs{mt}")
                        nc.vector.reciprocal(rs, se)
                        wt = soft.tile([128, O], f32, tag=f"wt{mt}")
                        nc.vector.tensor_scalar_mul(wt, e, rs)
                        nc.sync.dma_start(
                            out=w_d.rearrange("(mt p) o -> p mt o", p=128)[:, mt, :],
                            in_=wt,
                        )
                    if repeat > 1:
                        nc.sync.dma_start(out=it_d[:], in_=cnt)

    nc.finalize()
    _BUILD_CACHE[key] = nc
    return nc


def prepare_in_maps(v, q, box_mask, Wv, bv, Wq, bq, Wn, bn, Wl, bl, mm_bf16=False, vt_host_pad=False):
    """Host-side prep: transposes, |Wl| fold + sign permutation, mask->additive."""
    v = np.ascontiguousarray(np.asarray(v, dtype=np.float32))
    q = np.asarray(q, dtype=np.float32).reshape(B, KT, QD)
    box_mask = np.asarray(box_mask)
    Wv = np.asarray(Wv, dtype=np.float32)
    Wq = np.asarray(Wq, dtype=np.float32)
    Wn = np.asarray(Wn, dtype=np.float32)
    Wl = np.asarray(Wl, dtype=np.float32)
    bv = np.asarray(bv, dtype=np.float32)
    bq = np.asarray(bq, dtype=np.float32)
    bn = np.asarray(bn, dtype=np.float32)
    # bl is dropped: softmax is shift-invariant.

    wl = Wl.reshape(H)
    pos = np.flatnonzero(wl > 0)
    rest = np.flatnonzero(~(wl > 0))
    perm = np.concatenate([pos, rest])
    gpos = int(pos.size)
    wn2t = np.ascontiguousarray((Wn * np.abs(wl)[:, None])[perm].T)  # [h, g']
    if mm_bf16:
        import ml_dtypes
        wn2t = wn2t.astype(ml_dtypes.bfloat16)
    use_bn = bool(np.any(bn != 0.0))
    bn2 = np.ascontiguousarray((bn * np.abs(wl))[perm].reshape(1, H))

    wvt = np.ascontiguousarray(Wv.T)  # [VD, H]
    wqt = np.ascontiguousarray(Wq.T)  # [QD, H]
    maskadd = np.where(box_mask > 0, np.float32(0.0), np.float32(NEG)).astype(np.float32)

    in_maps = []
    for b in range(B):
        if vt_host_pad:
            vtp = np.zeros((VD, OPAD), dtype=np.float32)
            vtp[:, :O] = v[b].T
        else:
            vtp = np.ascontiguousarray(v[b].T)
        qtp = np.ascontiguousarray(q[b].T)
        m = {
            "vtp": vtp,
            "qtp": qtp,
            "wvt": wvt,
            "wqt": wqt,
            "wnt": wn2t,
            "bv": bv.reshape(H, 1),
            "bq": bq.reshape(H, 1),
            "maskadd": np.ascontiguousarray(maskadd[b : b + 1]),
        }
        if use_bn:
            m["bn2"] = bn2
        in_maps.append(m)
    return in_maps, gpos, use_bn


def kernel(**inputs) -> np.ndarray:
    in_maps, gpos, use_bn = prepare_in_maps(
        inputs["v"], inputs["q"], inputs["box_mask"],
        inputs["Wv"], inputs["bv"], inputs["Wq"], inputs["bq"],
        inputs["Wn"], inputs["bn"], inputs["Wl"], inputs["bl"],
    )
    nc = build_nc(gpos, use_bn, repeat=1)
    res = run_bass_kernel_spmd(nc, in_maps, core_ids=list(range(N_CORES)))
    out = np.stack(
        [res.results[b]["w"].reshape(K, T, O) for b in range(B)]
    )
    return np.ascontiguousarray(out.astype(np.float32))


# revision 10
# speedup vs baseline: 1094.0365x; 1.1534x over previous
"""Trainium2 Bass kernel for nn_Att_3_layer1 (dense attention scorer + masked softmax).

Math (per batch b):
    v_proj = relu(v @ Wv^T + bv)                    [O, H]
    q_proj = relu(q @ Wq^T + bq)                    [KT, H]
    joint  = v_proj[o] * q_proj[kt]  (elementwise)  per (kt, o) pair
    pre    = joint @ Wn^T + bn                      [KT, O, G]
    logits = relu(pre) @ Wl^T + bl                  [KT, O]
    w      = softmax(where(mask, logits, -1e9), axis=o)

Sharding: data-parallel over batch (B=8) across 8 NeuronCores; weights replicated.

Device-side algorithm (per core, one batch element):
  - All matmuls run in fp32r (1 cycle/row on the PE at N>=256, ~1e-4 rel err).
  - Stage 1: v_projT[h, o] = relu(Wv @ v^T + bv), o padded to 256 for full-rate fp32r.
  - Stage 2: q_projT[h, kt] = relu(Wq @ q^T + bq).
  - Per o (100 iterations): J_o[h, kt] = q_projT * v_projT[:, o] (DVE per-partition
    scalar mul, output rounded to fp32r); then pre[kt, g] = J_o^T @ Wn'^T as 8
    accumulating 128x128x512 fp32r matmuls.
  - Wl contraction trick: relu(x)*wl summed over g == sum(relu(x*|wl|)*sign(wl)).
    |Wl| is folded into Wn's rows on the host and g is permuted so positive-sign
    columns come first; the logits reduce to two relu+sum passes over column
    blocks of PSUM, done with activation(Relu, accum_out=...) on ScalarE (3 of 4
    blocks) and tensor_scalar(max0, accum_out=...) on VectorE (1 of 4), so the
    reduction runs in parallel with the PE matmul stream.
  - bl drops out (softmax shift invariance). bn is zero in this problem instance;
    if nonzero it is folded in via an extra K=1 ones-row matmul.
  - Masked softmax over o: mask converted host-side to additive 0/-1e9, broadcast
    to 128 partitions with a K=1 ones matmul; rowmax via tensor_reduce(negate),
    exp+sum in one activation(Exp, accum_out), reciprocal, scale.
"""

import os
import numpy as np

import concourse.bass as bass
import concourse.tile as tile
from concourse import bacc, mybir
from concourse.bass_utils import run_bass_kernel_spmd

B, K, T, O = 8, 4, 64, 100
VD, QD, H = 2048, 768, 512
KT = K * T            # 256
OPAD = 256            # fp32r matmuls need moving dim >= 256 for 1 cycle/row
NEG = -1e9
N_CORES = 8

f32 = mybir.dt.float32
f32r = mybir.dt.float32r

HT = H // 128     # 4
VT = VD // 128    # 16
QT = QD // 128    # 6
MT = KT // 128    # 2

_BUILD_CACHE: dict = {}


def build_nc(gpos: int, use_bn: bool, repeat: int = 1, *, dve_jobs=(0, 2), scr_bf16=False, po_bufs=2, j_bufs=3, scr_bufs=2, mm_bf16=False, vt_host_pad=False, chunked_wv=True):
    """Build the Bass program. gpos = number of positive-sign Wl columns after
    the host-side permutation; repeat wraps the whole body in a For_i for
    timing runs."""
    key = (gpos, use_bn, repeat, tuple(dve_jobs), scr_bf16, po_bufs, j_bufs, scr_bufs, mm_bf16, vt_host_pad, chunked_wv)
    if key in _BUILD_CACHE:
        return _BUILD_CACHE[key]

    nc = bacc.Bacc("TRN2", target_bir_lowering=False, debug=True)

    v_d = nc.dram_tensor("vtp", [VD, OPAD if vt_host_pad else O], f32r, kind="ExternalInput")
    q_d = nc.dram_tensor("qtp", [QD, KT], f32r, kind="ExternalInput")
    wvt_d = nc.dram_tensor("wvt", [VD, H], f32r, kind="ExternalInput")
    wqt_d = nc.dram_tensor("wqt", [QD, H], f32r, kind="ExternalInput")
    mmdt = mybir.dt.bfloat16 if mm_bf16 else f32r
    wnt_d = nc.dram_tensor("wnt", [H, H], mmdt, kind="ExternalInput")
    bv_d = nc.dram_tensor("bv", [H, 1], f32, kind="ExternalInput")
    bq_d = nc.dram_tensor("bq", [H, 1], f32, kind="ExternalInput")
    mk_d = nc.dram_tensor("maskadd", [1, O], f32, kind="ExternalInput")
    bn_d = nc.dram_tensor("bn2", [1, H], f32r, kind="ExternalInput") if use_bn else None
    w_d = nc.dram_tensor("w", [KT, O], f32, kind="ExternalOutput")
    it_d = nc.dram_tensor("iters", [1, 1], f32, kind="ExternalOutput") if repeat > 1 else None

    Relu = mybir.ActivationFunctionType.Relu
    Exp = mybir.ActivationFunctionType.Exp

    # stage-5 reduce jobs: (mt, lo, hi, kind, engine); dest tile chosen per job
    jobs = []
    for mt in range(MT):
        if gpos > 0:
            jobs.append((mt, 0, gpos, "P"))
        if gpos < H:
            jobs.append((mt, gpos, H, "N"))
    dve_jobs = set(dve_jobs)

    with tile.TileContext(nc) as tc:
        import contextlib

        with contextlib.ExitStack() as stk:
            wpool = stk.enter_context(tc.tile_pool(name="wpool", bufs=1))
            jpool = stk.enter_context(tc.tile_pool(name="jpool", bufs=j_bufs))
            scrpool = stk.enter_context(tc.tile_pool(name="scrpool", bufs=scr_bufs))

            # persistent tiles (addresses fixed; loaded inside the loop body)
            wvt_s = wpool.tile([128, VT, H], f32r, tag="wvt")
            wqt_s = wpool.tile([128, QT, H], f32r, tag="wqt")
            wnt_s = wpool.tile([128, HT, H], mmdt, tag="wnt")
            vt_s = wpool.tile([128, VT, OPAD], f32r, tag="vt")
            qt_s = wpool.tile([128, QT, KT], f32r, tag="qt")
            bv_s = wpool.tile([128, HT, 1], f32, tag="bv")
            bq_s = wpool.tile([128, HT, 1], f32, tag="bq")
            mk_s = wpool.tile([1, O], f32, tag="mk")
            ones1 = wpool.tile([1, 128], f32, tag="ones1")
            v_projT = wpool.tile([128, HT, O], f32, tag="vproj")
            q_projT = wpool.tile([128, HT, KT], mybir.dt.bfloat16 if mm_bf16 else f32, tag="qproj")
            m128 = wpool.tile([128, O], f32, tag="m128")
            lgP = [wpool.tile([128, O], f32, tag=f"lgP{mt}", name=f"lgP{mt}") for mt in range(MT)]
            lgN = [wpool.tile([128, O], f32, tag=f"lgN{mt}", name=f"lgN{mt}") for mt in range(MT)]
            if use_bn:
                ones1r = wpool.tile([1, 128], f32r, tag="ones1r")
                bn_s = wpool.tile([1, H], f32r, tag="bn")

            nc.vector.memset(ones1, 1.0)
            if not vt_host_pad:
                for _k in range(VT):
                    nc.gpsimd.memset(vt_s[:, _k, O:OPAD].bitcast(mybir.dt.uint32), 0)
            if repeat > 1:
                cnt = wpool.tile([1, 1], f32, tag="cnt")
                nc.vector.memset(cnt, 0.0)
            if use_bn:
                nc.gpsimd.memset(ones1r, 1.0)
            if gpos == 0:
                for mt in range(MT):
                    nc.gpsimd.memset(lgP[mt], 0.0)
            if gpos == H:
                for mt in range(MT):
                    nc.gpsimd.memset(lgN[mt], 0.0)

            loop = (
                tc.For_i(
                    0,
                    repeat,
                    1,
                    hint_engines=(
                        mybir.EngineType.PE,
                        mybir.EngineType.Activation,
                        mybir.EngineType.DVE,
                        mybir.EngineType.SP,
                    ),
                )
                if repeat > 1
                else contextlib.nullcontext()
            )
            with loop:
                if repeat > 1:
                    nc.vector.tensor_scalar_add(cnt, cnt, 1.0)
                # ---- input DMAs: stage-2 deps first so the PE gets work
                # early; the big WvT load is split into 4 chunks so stage-1
                # matmuls pipeline with the transfer
                nc.sync.dma_start(out=wqt_s, in_=wqt_d.rearrange("(t p) h -> p t h", p=128))
                nc.sync.dma_start(out=qt_s, in_=q_d.rearrange("(t p) kt -> p t kt", p=128))
                nc.sync.dma_start(out=bq_s, in_=bq_d.rearrange("(t p) x -> p t x", p=128))
                nc.sync.dma_start(out=bv_s, in_=bv_d.rearrange("(t p) x -> p t x", p=128))
                if vt_host_pad:
                    nc.sync.dma_start(out=vt_s, in_=v_d.rearrange("(t p) o -> p t o", p=128))
                else:
                    nc.sync.dma_start(out=vt_s[:, :, :O], in_=v_d.rearrange("(t p) o -> p t o", p=128))
                wvt_src = wvt_d.rearrange("(t p) h -> p t h", p=128)
                VCH = 4 if chunked_wv else 1
                for c in range(VCH):
                    nc.sync.dma_start(
                        out=wvt_s[:, c * (VT // VCH) : (c + 1) * (VT // VCH), :],
                        in_=wvt_src[:, c * (VT // VCH) : (c + 1) * (VT // VCH), :],
                    )
                nc.sync.dma_start(out=wnt_s, in_=wnt_d.rearrange("(t p) g -> p t g", p=128))
                nc.sync.dma_start(out=mk_s, in_=mk_d[:])
                if use_bn:
                    nc.sync.dma_start(out=bn_s, in_=bn_d[:])

                # ---- stage 1 + 2: projections (PSUM pool scoped so the o-loop
                # pool can use all banks afterwards)
                with tc.tile_pool(name="pre_ps", bufs=1, space="PSUM") as pre_ps:
                    for m in range(HT):
                        pq = pre_ps.tile([128, KT], f32, tag=f"pq{m}")
                        for k in range(QT):
                            nc.tensor.matmul(
                                pq[:],
                                wqt_s[:, k, m * 128 : (m + 1) * 128],
                                qt_s[:, k, :],
                                start=(k == 0),
                                stop=(k == QT - 1),
                            )
                        nc.scalar.activation(
                            out=q_projT[:, m, :], in_=pq[:], func=Relu,
                            bias=bq_s[:, m, :], scale=1.0,
                        )
                    # stage 1: k-tiles grouped by WvT DMA chunk so matmuls can
                    # start while later chunks are still in flight
                    pvs = [pre_ps.tile([128, OPAD], f32, tag=f"pv{m}", name=f"pv{m}") for m in range(HT)]
                    for c in range(VCH):
                        for m in range(HT):
                            for k in range(c * (VT // VCH), (c + 1) * (VT // VCH)):
                                nc.tensor.matmul(
                                    pvs[m][:],
                                    wvt_s[:, k, m * 128 : (m + 1) * 128],
                                    vt_s[:, k, :],
                                    start=(k == 0),
                                    stop=(k == VT - 1),
                                )
                    for m in range(HT):
                        nc.scalar.activation(
                            out=v_projT[:, m, :], in_=pvs[m][:, :O], func=Relu,
                            bias=bv_s[:, m, :], scale=1.0,
                        )

                with tc.tile_pool(name="o_ps", bufs=1, space="PSUM") as pso:
                    # mask broadcast to all 128 partitions via K=1 ones matmul
                    pm = pso.tile([128, O], f32, tag="pm")
                    nc.tensor.matmul(pm[:], ones1[:], mk_s[:], start=True, stop=True)
                    nc.vector.tensor_copy(m128, pm[:])

                    # ---- main o-loop
                    for o in range(O):
                        js = [
                            jpool.tile([128, KT], mmdt, tag=f"j{k}", name=f"j{k}_{o}")
                            for k in range(HT)
                        ]
                        for k in range(HT):
                            nc.vector.tensor_scalar_mul(
                                js[k], q_projT[:, k, :], v_projT[:, k, o : o + 1]
                            )
                        for mt in range(MT):
                            po = pso.tile([128, H], f32, tag=f"po{mt}", bufs=po_bufs)
                            for k in range(HT):
                                nc.tensor.matmul(
                                    po[:],
                                    js[k][:, mt * 128 : (mt + 1) * 128],
                                    wnt_s[:, k, :],
                                    start=(k == 0),
                                    stop=(k == HT - 1 and not use_bn),
                                )
                            if use_bn:
                                nc.tensor.matmul(
                                    po[:], ones1r[:], bn_s[:], start=False, stop=True
                                )
                            for ji, (jmt, lo, hi, kind) in enumerate(jobs):
                                if jmt != mt:
                                    continue
                                dest = (lgP if kind == "P" else lgN)[mt][:, o : o + 1]
                                scr = scrpool.tile(
                                    [128, hi - lo],
                                    mybir.dt.bfloat16 if scr_bf16 else f32,
                                    tag=f"scr{ji}",
                                    name=f"scr{ji}_{o}",
                                )
                                if ji in dve_jobs:
                                    nc.vector.tensor_scalar(
                                        out=scr,
                                        in0=po[:, lo:hi],
                                        scalar1=0.0,
                                        scalar2=None,
                                        op0=mybir.AluOpType.max,
                                        op1=mybir.AluOpType.add,
                                        accum_out=dest,
                                    )
                                else:
                                    nc.scalar.activation(
                                        out=scr, in_=po[:, lo:hi], func=Relu,
                                        accum_out=dest,
                                    )

                # ---- masked softmax over o (free dim), per kt-tile
                with tc.tile_pool(name="soft", bufs=1) as soft:
                    for mt in range(MT):
                        lg = soft.tile([128, O], f32, tag=f"lg{mt}")
                        nc.vector.tensor_sub(lg, lgP[mt], lgN[mt])
                        lgm = soft.tile([128, O], f32, tag=f"lgm{mt}")
                        nc.vector.tensor_add(lgm, lg, m128)
                        mx = soft.tile([128, 1], f32, tag=f"mx{mt}")
                        nc.vector.tensor_reduce(
                            out=mx, in_=lgm, axis=mybir.AxisListType.X,
                            op=mybir.AluOpType.max, negate=True,
                        )
                        e = soft.tile([128, O], f32, tag=f"e{mt}")
                        se = soft.tile([128, 1], f32, tag=f"se{mt}")
                        nc.scalar.activation(
                            out=e, in_=lgm, func=Exp, bias=mx, scale=1.0, accum_out=se
                        )
                        rs = soft.tile([128, 1], f32, tag=f"rs{mt}")
                        nc.vector.reciprocal(rs, se)
                        wt = soft.tile([128, O], f32, tag=f"wt{mt}")
                        nc.vector.tensor_scalar_mul(wt, e, rs)
                        nc.sync.dma_start(
                            out=w_d.rearrange("(mt p) o -> p mt o", p=128)[:, mt, :],
                            in_=wt,
                        )
                    if repeat > 1:
                        nc.sync.dma_start(out=it_d[:], in_=cnt)

    nc.finalize()
    _BUILD_CACHE[key] = nc
    return nc


def prepare_in_maps(v, q, box_mask, Wv, bv, Wq, bq, Wn, bn, Wl, bl, mm_bf16=False, vt_host_pad=False):
    """Host-side prep: transposes, |Wl| fold + sign permutation, mask->additive."""
    v = np.ascontiguousarray(np.asarray(v, dtype=np.float32))
    q = np.asarray(q, dtype=np.float32).reshape(B, KT, QD)
    box_mask = np.asarray(box_mask)
    Wv = np.asarray(Wv, dtype=np.float32)
    Wq = np.asarray(Wq, dtype=np.float32)
    Wn = np.asarray(Wn, dtype=np.float32)
    Wl = np.asarray(Wl, dtype=np.float32)
    bv = np.asarray(bv, dtype=np.float32)
    bq = np.asarray(bq, dtype=np.float32)
    bn = np.asarray(bn, dtype=np.float32)
    # bl is dropped: softmax is shift-invariant.

    wl = Wl.reshape(H)
    pos = np.flatnonzero(wl > 0)
    rest = np.flatnonzero(~(wl > 0))
    perm = np.concatenate([pos, rest])
    gpos = int(pos.size)
    wn2t = np.ascontiguousarray((Wn * np.abs(wl)[:, None])[perm].T)  # [h, g']
    if mm_bf16:
        import ml_dtypes
        wn2t = wn2t.astype(ml_dtypes.bfloat16)
    use_bn = bool(np.any(bn != 0.0))
    bn2 = np.ascontiguousarray((bn * np.abs(wl))[perm].reshape(1, H))

    wvt = np.ascontiguousarray(Wv.T)  # [VD, H]
    wqt = np.ascontiguousarray(Wq.T)  # [QD, H]
    maskadd = np.where(box_mask > 0, np.float32(0.0), np.float32(NEG)).astype(np.float32)

    in_maps = []
    for b in range(B):
        if vt_host_pad:
            vtp = np.zeros((VD, OPAD), dtype=np.float32)
            vtp[:, :O] = v[b].T
        else:
            vtp = np.ascontiguousarray(v[b].T)
        qtp = np.ascontiguousarray(q[b].T)
        m = {
            "vtp": vtp,
            "qtp": qtp,
            "wvt": wvt,
            "wqt": wqt,
            "wnt": wn2t,
            "bv": bv.reshape(H, 1),
            "bq": bq.reshape(H, 1),
            "maskadd": np.ascontiguousarray(maskadd[b : b + 1]),
        }
        if use_bn:
            m["bn2"] = bn2
        in_maps.append(m)
    return in_maps, gpos, use_bn


def kernel(**inputs) -> np.ndarray:
    in_maps, gpos, use_bn = prepare_in_maps(
        inputs["v"], inputs["q"], inputs["box_mask"],
        inputs["Wv"], inputs["bv"], inputs["Wq"], inputs["bq"],
        inputs["Wn"], inputs["bn"], inputs["Wl"], inputs["bl"],
    )
    nc = build_nc(gpos, use_bn, repeat=1)
    res = run_bass_kernel_spmd(nc, in_maps, core_ids=list(range(N_CORES)))
    out = np.stack(
        [res.results[b]["w"].reshape(K, T, O) for b in range(B)]
    )
    return np.ascontiguousarray(out.astype(np.float32))


# revision 11
# speedup vs baseline: 1107.8142x; 1.0126x over previous
"""Trainium2 Bass kernel for nn_Att_3_layer1 (dense attention scorer + masked softmax).

Math (per batch b):
    v_proj = relu(v @ Wv^T + bv)                    [O, H]
    q_proj = relu(q @ Wq^T + bq)                    [KT, H]
    joint  = v_proj[o] * q_proj[kt]  (elementwise)  per (kt, o) pair
    pre    = joint @ Wn^T + bn                      [KT, O, G]
    logits = relu(pre) @ Wl^T + bl                  [KT, O]
    w      = softmax(where(mask, logits, -1e9), axis=o)

Sharding: data-parallel over batch (B=8) across 8 NeuronCores; weights replicated.

Device-side algorithm (per core, one batch element):
  - All matmuls run in fp32r (1 cycle/row on the PE at N>=256, ~1e-4 rel err).
  - Stage 1: v_projT[h, o] = relu(Wv @ v^T + bv), o padded to 256 for full-rate fp32r.
  - Stage 2: q_projT[h, kt] = relu(Wq @ q^T + bq).
  - Per o (100 iterations): J_o[h, kt] = q_projT * v_projT[:, o] (DVE per-partition
    scalar mul, output rounded to fp32r); then pre[kt, g] = J_o^T @ Wn'^T as 8
    accumulating 128x128x512 fp32r matmuls.
  - Wl contraction trick: relu(x)*wl summed over g == sum(relu(x*|wl|)*sign(wl)).
    |Wl| is folded into Wn's rows on the host and g is permuted so positive-sign
    columns come first; the logits reduce to two relu+sum passes over column
    blocks of PSUM, done with activation(Relu, accum_out=...) on ScalarE (3 of 4
    blocks) and tensor_scalar(max0, accum_out=...) on VectorE (1 of 4), so the
    reduction runs in parallel with the PE matmul stream.
  - bl drops out (softmax shift invariance). bn is zero in this problem instance;
    if nonzero it is folded in via an extra K=1 ones-row matmul.
  - Masked softmax over o: mask converted host-side to additive 0/-1e9, broadcast
    to 128 partitions with a K=1 ones matmul; rowmax via tensor_reduce(negate),
    exp+sum in one activation(Exp, accum_out), reciprocal, scale.
"""

import os
import numpy as np

import concourse.bass as bass
import concourse.tile as tile
from concourse import bacc, mybir
from concourse.bass_utils import run_bass_kernel_spmd

B, K, T, O = 8, 4, 64, 100
VD, QD, H = 2048, 768, 512
KT = K * T            # 256
OPAD = 256            # fp32r matmuls need moving dim >= 256 for 1 cycle/row
NEG = -1e9
N_CORES = 8

f32 = mybir.dt.float32
f32r = mybir.dt.float32r

HT = H // 128     # 4
VT = VD // 128    # 16
QT = QD // 128    # 6
MT = KT // 128    # 2

_BUILD_CACHE: dict = {}


def build_nc(gpos: int, use_bn: bool, repeat: int = 1, *, dve_jobs=(0, 2), scr_bf16=False, po_bufs=2, j_bufs=3, scr_bufs=2, mm_dt="f32r", vt_host_pad=False, chunked_wv=True):
    """Build the Bass program. gpos = number of positive-sign Wl columns after
    the host-side permutation; repeat wraps the whole body in a For_i for
    timing runs."""
    key = (gpos, use_bn, repeat, tuple(dve_jobs), scr_bf16, po_bufs, j_bufs, scr_bufs, mm_dt, vt_host_pad, chunked_wv)
    if key in _BUILD_CACHE:
        return _BUILD_CACHE[key]

    nc = bacc.Bacc("TRN2", target_bir_lowering=False, debug=True)

    v_d = nc.dram_tensor("vtp", [VD, OPAD if vt_host_pad else O], f32r, kind="ExternalInput")
    q_d = nc.dram_tensor("qtp", [QD, KT], f32r, kind="ExternalInput")
    wvt_d = nc.dram_tensor("wvt", [VD, H], f32r, kind="ExternalInput")
    wqt_d = nc.dram_tensor("wqt", [QD, H], f32r, kind="ExternalInput")
    mmdt = {"f32r": f32r, "bf16": mybir.dt.bfloat16, "f16": mybir.dt.float16}[mm_dt]
    wnt_d = nc.dram_tensor("wnt", [H, H], mmdt, kind="ExternalInput")
    bv_d = nc.dram_tensor("bv", [H, 1], f32, kind="ExternalInput")
    bq_d = nc.dram_tensor("bq", [H, 1], f32, kind="ExternalInput")
    mk_d = nc.dram_tensor("maskadd", [1, O], f32, kind="ExternalInput")
    bn_d = nc.dram_tensor("bn2", [1, H], f32r, kind="ExternalInput") if use_bn else None
    w_d = nc.dram_tensor("w", [KT, O], f32, kind="ExternalOutput")
    it_d = nc.dram_tensor("iters", [1, 1], f32, kind="ExternalOutput") if repeat > 1 else None

    Relu = mybir.ActivationFunctionType.Relu
    Exp = mybir.ActivationFunctionType.Exp

    # stage-5 reduce jobs: (mt, lo, hi, kind, engine); dest tile chosen per job
    jobs = []
    for mt in range(MT):
        if gpos > 0:
            jobs.append((mt, 0, gpos, "P"))
        if gpos < H:
            jobs.append((mt, gpos, H, "N"))
    dve_jobs = set(dve_jobs)

    with tile.TileContext(nc) as tc:
        import contextlib

        with contextlib.ExitStack() as stk:
            wpool = stk.enter_context(tc.tile_pool(name="wpool", bufs=1))
            jpool = stk.enter_context(tc.tile_pool(name="jpool", bufs=j_bufs))
            scrpool = stk.enter_context(tc.tile_pool(name="scrpool", bufs=scr_bufs))

            # persistent tiles (addresses fixed; loaded inside the loop body)
            wvt_s = wpool.tile([128, VT, H], f32r, tag="wvt")
            wqt_s = wpool.tile([128, QT, H], f32r, tag="wqt")
            wnt_s = wpool.tile([128, HT, H], mmdt, tag="wnt")
            vt_s = wpool.tile([128, VT, OPAD], f32r, tag="vt")
            qt_s = wpool.tile([128, QT, KT], f32r, tag="qt")
            bv_s = wpool.tile([128, HT, 1], f32, tag="bv")
            bq_s = wpool.tile([128, HT, 1], f32, tag="bq")
            mk_s = wpool.tile([1, O], f32, tag="mk")
            ones1 = wpool.tile([1, 128], f32, tag="ones1")
            v_projT = wpool.tile([128, HT, O], f32, tag="vproj")
            q_projT = wpool.tile([128, HT, KT], f32 if mm_dt == "f32r" else mmdt, tag="qproj")
            m128 = wpool.tile([128, O], f32, tag="m128")
            lgP = [wpool.tile([128, O], f32, tag=f"lgP{mt}", name=f"lgP{mt}") for mt in range(MT)]
            lgN = [wpool.tile([128, O], f32, tag=f"lgN{mt}", name=f"lgN{mt}") for mt in range(MT)]
            if use_bn:
                ones1r = wpool.tile([1, 128], f32r, tag="ones1r")
                bn_s = wpool.tile([1, H], f32r, tag="bn")

            nc.vector.memset(ones1, 1.0)
            if not vt_host_pad:
                for _k in range(VT):
                    nc.gpsimd.memset(vt_s[:, _k, O:OPAD].bitcast(mybir.dt.uint32), 0)
            if repeat > 1:
                cnt = wpool.tile([1, 1], f32, tag="cnt")
                nc.vector.memset(cnt, 0.0)
            if use_bn:
                nc.gpsimd.memset(ones1r, 1.0)
            if gpos == 0:
                for mt in range(MT):
                    nc.gpsimd.memset(lgP[mt], 0.0)
            if gpos == H:
                for mt in range(MT):
                    nc.gpsimd.memset(lgN[mt], 0.0)

            loop = (
                tc.For_i(
                    0,
                    repeat,
                    1,
                    hint_engines=(
                        mybir.EngineType.PE,
                        mybir.EngineType.Activation,
                        mybir.EngineType.DVE,
                        mybir.EngineType.SP,
                    ),
                )
                if repeat > 1
                else contextlib.nullcontext()
            )
            with loop:
                if repeat > 1:
                    nc.vector.tensor_scalar_add(cnt, cnt, 1.0)
                # ---- input DMAs: stage-2 deps first so the PE gets work
                # early; the big WvT load is split into 4 chunks so stage-1
                # matmuls pipeline with the transfer
                nc.sync.dma_start(out=wqt_s, in_=wqt_d.rearrange("(t p) h -> p t h", p=128))
                nc.sync.dma_start(out=qt_s, in_=q_d.rearrange("(t p) kt -> p t kt", p=128))
                nc.sync.dma_start(out=bq_s, in_=bq_d.rearrange("(t p) x -> p t x", p=128))
                nc.sync.dma_start(out=bv_s, in_=bv_d.rearrange("(t p) x -> p t x", p=128))
                if vt_host_pad:
                    nc.sync.dma_start(out=vt_s, in_=v_d.rearrange("(t p) o -> p t o", p=128))
                else:
                    nc.sync.dma_start(out=vt_s[:, :, :O], in_=v_d.rearrange("(t p) o -> p t o", p=128))
                wvt_src = wvt_d.rearrange("(t p) h -> p t h", p=128)
                VCH = 4 if chunked_wv else 1
                for c in range(VCH):
                    nc.sync.dma_start(
                        out=wvt_s[:, c * (VT // VCH) : (c + 1) * (VT // VCH), :],
                        in_=wvt_src[:, c * (VT // VCH) : (c + 1) * (VT // VCH), :],
                    )
                nc.sync.dma_start(out=wnt_s, in_=wnt_d.rearrange("(t p) g -> p t g", p=128))
                nc.sync.dma_start(out=mk_s, in_=mk_d[:])
                if use_bn:
                    nc.sync.dma_start(out=bn_s, in_=bn_d[:])

                # ---- stage 1 + 2: projections (PSUM pool scoped so the o-loop
                # pool can use all banks afterwards)
                with tc.tile_pool(name="pre_ps", bufs=1, space="PSUM") as pre_ps:
                    for m in range(HT):
                        pq = pre_ps.tile([128, KT], f32, tag=f"pq{m}")
                        for k in range(QT):
                            nc.tensor.matmul(
                                pq[:],
                                wqt_s[:, k, m * 128 : (m + 1) * 128],
                                qt_s[:, k, :],
                                start=(k == 0),
                                stop=(k == QT - 1),
                            )
                        nc.scalar.activation(
                            out=q_projT[:, m, :], in_=pq[:], func=Relu,
                            bias=bq_s[:, m, :], scale=1.0,
                        )
                    # stage 1: k-tiles grouped by WvT DMA chunk so matmuls can
                    # start while later chunks are still in flight
                    pvs = [pre_ps.tile([128, OPAD], f32, tag=f"pv{m}", name=f"pv{m}") for m in range(HT)]
                    for c in range(VCH):
                        for m in range(HT):
                            for k in range(c * (VT // VCH), (c + 1) * (VT // VCH)):
                                nc.tensor.matmul(
                                    pvs[m][:],
                                    wvt_s[:, k, m * 128 : (m + 1) * 128],
                                    vt_s[:, k, :],
                                    start=(k == 0),
                                    stop=(k == VT - 1),
                                )
                    for m in range(HT):
                        nc.scalar.activation(
                            out=v_projT[:, m, :], in_=pvs[m][:, :O], func=Relu,
                            bias=bv_s[:, m, :], scale=1.0,
                        )

                with tc.tile_pool(name="o_ps", bufs=1, space="PSUM") as pso:
                    # mask broadcast to all 128 partitions via K=1 ones matmul
                    pm = pso.tile([128, O], f32, tag="pm")
                    nc.tensor.matmul(pm[:], ones1[:], mk_s[:], start=True, stop=True)
                    nc.vector.tensor_copy(m128, pm[:])

                    # ---- main o-loop
                    for o in range(O):
                        js = [
                            jpool.tile([128, KT], mmdt, tag=f"j{k}", name=f"j{k}_{o}")
                            for k in range(HT)
                        ]
                        for k in range(HT):
                            nc.vector.tensor_scalar_mul(
                                js[k], q_projT[:, k, :], v_projT[:, k, o : o + 1]
                            )
                        for mt in range(MT):
                            po = pso.tile([128, H], f32, tag=f"po{mt}", bufs=po_bufs)
                            for k in range(HT):
                                nc.tensor.matmul(
                                    po[:],
                                    js[k][:, mt * 128 : (mt + 1) * 128],
                                    wnt_s[:, k, :],
                                    start=(k == 0),
                                    stop=(k == HT - 1 and not use_bn),
                                )
                            if use_bn:
                                nc.tensor.matmul(
                                    po[:], ones1r[:], bn_s[:], start=False, stop=True
                                )
                            for ji, (jmt, lo, hi, kind) in enumerate(jobs):
                                if jmt != mt:
                                    continue
                                dest = (lgP if kind == "P" else lgN)[mt][:, o : o + 1]
                                scr = scrpool.tile(
                                    [128, hi - lo],
                                    mybir.dt.bfloat16 if scr_bf16 else f32,
                                    tag=f"scr{ji}",
                                    name=f"scr{ji}_{o}",
                                )
                                if ji in dve_jobs:
                                    nc.vector.tensor_scalar(
                                        out=scr,
                                        in0=po[:, lo:hi],
                                        scalar1=0.0,
                                        scalar2=None,
                                        op0=mybir.AluOpType.max,
                                        op1=mybir.AluOpType.add,
                                        accum_out=dest,
                                    )
                                else:
                                    nc.scalar.activation(
                                        out=scr, in_=po[:, lo:hi], func=Relu,
                                        accum_out=dest,
                                    )

                # ---- masked softmax over o (free dim), per kt-tile
                with tc.tile_pool(name="soft", bufs=1) as soft:
                    for mt in range(MT):
                        lg = soft.tile([128, O], f32, tag=f"lg{mt}")
                        nc.vector.tensor_sub(lg, lgP[mt], lgN[mt])
                        lgm = soft.tile([128, O], f32, tag=f"lgm{mt}")
                        nc.vector.tensor_add(lgm, lg, m128)
                        mx = soft.tile([128, 1], f32, tag=f"mx{mt}")
                        nc.vector.tensor_reduce(
                            out=mx, in_=lgm, axis=mybir.AxisListType.X,
                            op=mybir.AluOpType.max, negate=True,
                        )
                        e = soft.tile([128, O], f32, tag=f"e{mt}")
                        se = soft.tile([128, 1], f32, tag=f"se{mt}")
                        nc.scalar.activation(
                            out=e, in_=lgm, func=Exp, bias=mx, scale=1.0, accum_out=se
                        )
                        rs = soft.tile([128, 1], f32, tag=f"rs{mt}")
                        nc.vector.reciprocal(rs, se)
                        wt = soft.tile([128, O], f32, tag=f"wt{mt}")
                        nc.vector.tensor_scalar_mul(wt, e, rs)
                        nc.sync.dma_start(
                            out=w_d.rearrange("(mt p) o -> p mt o", p=128)[:, mt, :],
                            in_=wt,
                        )
                    if repeat > 1:
                        nc.sync.dma_start(out=it_d[:], in_=cnt)

    nc.finalize()
    _BUILD_CACHE[key] = nc
    return nc


def prepare_in_maps(v, q, box_mask, Wv, bv, Wq, bq, Wn, bn, Wl, bl, mm_dt="f32r", vt_host_pad=False):
    """Host-side prep: transposes, |Wl| fold + sign permutation, mask->additive."""
    v = np.ascontiguousarray(np.asarray(v, dtype=np.float32))
    q = np.asarray(q, dtype=np.float32).reshape(B, KT, QD)
    box_mask = np.asarray(box_mask)
    Wv = np.asarray(Wv, dtype=np.float32)
    Wq = np.asarray(Wq, dtype=np.float32)
    Wn = np.asarray(Wn, dtype=np.float32)
    Wl = np.asarray(Wl, dtype=np.float32)
    bv = np.asarray(bv, dtype=np.float32)
    bq = np.asarray(bq, dtype=np.float32)
    bn = np.asarray(bn, dtype=np.float32)
    # bl is dropped: softmax is shift-invariant.

    wl = Wl.reshape(H)
    pos = np.flatnonzero(wl > 0)
    rest = np.flatnonzero(~(wl > 0))
    perm = np.concatenate([pos, rest])
    gpos = int(pos.size)
    wn2t = np.ascontiguousarray((Wn * np.abs(wl)[:, None])[perm].T)  # [h, g']
    if mm_dt == "bf16":
        import ml_dtypes
        wn2t = wn2t.astype(ml_dtypes.bfloat16)
    elif mm_dt == "f16":
        wn2t = wn2t.astype(np.float16)
    use_bn = bool(np.any(bn != 0.0))
    bn2 = np.ascontiguousarray((bn * np.abs(wl))[perm].reshape(1, H))

    wvt = np.ascontiguousarray(Wv.T)  # [VD, H]
    wqt = np.ascontiguousarray(Wq.T)  # [QD, H]
    maskadd = np.where(box_mask > 0, np.float32(0.0), np.float32(NEG)).astype(np.float32)

    in_maps = []
    for b in range(B):
        if vt_host_pad:
            vtp = np.zeros((VD, OPAD), dtype=np.float32)
            vtp[:, :O] = v[b].T
        else:
            vtp = np.ascontiguousarray(v[b].T)
        qtp = np.ascontiguousarray(q[b].T)
        m = {
            "vtp": vtp,
            "qtp": qtp,
            "wvt": wvt,
            "wqt": wqt,
            "wnt": wn2t,
            "bv": bv.reshape(H, 1),
            "bq": bq.reshape(H, 1),
            "maskadd": np.ascontiguousarray(maskadd[b : b + 1]),
        }
        if use_bn:
            m["bn2"] = bn2
        in_maps.append(m)
    return in_maps, gpos, use_bn


def kernel(**inputs) -> np.ndarray:
    in_maps, gpos, use_bn = prepare_in_maps(
        inputs["v"], inputs["q"], inputs["box_mask"],
        inputs["Wv"], inputs["bv"], inputs["Wq"], inputs["bq"],
        inputs["Wn"], inputs["bn"], inputs["Wl"], inputs["bl"],
    )
    nc = build_nc(gpos, use_bn, repeat=1)
    res = run_bass_kernel_spmd(nc, in_maps, core_ids=list(range(N_CORES)))
    out = np.stack(
        [res.results[b]["w"].reshape(K, T, O) for b in range(B)]
    )
    return np.ascontiguousarray(out.astype(np.float32))
